# revision 31
# baseline (speedup 1.0000x reference)
# Multi-head attention (b=2, n=2048, d_model=1024, 16 heads) on 8 NeuronCores.
#
# Sharding: core c = (batch b, head-group g) with b = c//4, g = c%4.
# Each core handles 1 batch element and 4 heads (256 channels), computing a
# partial output projection; the host sums the 4 group-partials per batch and
# adds b_O.
#
# The kernel is ScalarE-bound: softmax needs exp on n^2*h/8 = 16.8M elements
# per core = 128 x [128,1024] ACT instructions ~= 137us. Everything else is
# scheduled to hide under that stream:
#
#  - Prologue: inputs are host-prepacked into the exact SBUF tile layouts
#    (weights k-major, x block-major [p, (blk, k, 512)]) so every tensor is
#    ONE contiguous 2D DMA with 4-8KB descriptors, issued on a single queue
#    in strict need-order (wk, x-blk0, wq, wv, bb, x-blk1, bv, blk2, blk3,
#    wot) - the 16 DMA engines drain one queue FIFO at full HBM bandwidth so
#    first-needed tensors land first. PE pre-warms on dummy matmuls (its
#    p-state ramps 0.65->2.4GHz only after ~3us of continuous execution);
#    ACT pre-loads the exp table. First exp fires at ~14us.
#  - Heads processed in PAIRS (cs in {0,1}; rows 0:64 / 64:128 of qt/kt[cs]).
#    The two score matmuls of a pair auto-derive PE row-tiles (0,0)/(64,0)
#    and co-stream. st pair-packed [128, 1024] in PSUM, double-buffered; ONE
#    [128,1024] Exp per (pair, m-slice) with the 1/8 scale folded in, output
#    directly in fp8e4.
#  - A*V runs in fp8 DoubleRow (Ko=2 packs consecutive m-slices). V stored
#    fp8 as v4p[mp] = [128, (ko=2, h=4, 72)] with a ones column at offset 64
#    (softmax denominators fall out of PSUM row 64 for free).
#  - A*V + end-of-segment normalize are emitted with a depth-2 deferral
#    queue: PE meets each A*V ~3 exp-slices after its inputs finished, so
#    the in-order PE pipe never waits on ScalarE (a PE stall both starves
#    the next scores and resets the p-state ramp, halving matmul speed).
#  - Normalize: DVE copy drains ot (frees the PSUM bank), the denominator
#    row transposes to [128,4] via SBUF-SBUF DMA so the exact DVE reciprocal
#    is ~free ([1,512] would cost 3.3us), transposes back, GpSimd broadcasts
#    across partitions, DVE multiplies into osb. All off the exp critical
#    path.
#  - Q/K/V/O projection chains are deadline-paced fillers in segment slots,
#    sized so each segment's PE work fits under its 17.1us exp window; Q/K
#    chains split into 4-matmul halves so a filler delays the score stream
#    by <1us. V(14)/V(15) spill into seg 1 (their A*V pops late enough).
#  - O-projection: both cs halves accumulate on ONE psum bank, single DVE
#    drain to f16, streamed out on the sync queue. Chunk 3 (only computable
#    after the last exp) pre-runs its cs0 halves inside seg 7 and finishes
#    with 4 add-units + 4 fused units behind the final normalize.
import ml_dtypes
import numpy as np

import concourse.bass as bass
import concourse.bacc as bacc
import concourse.tile as tile
from concourse import mybir
from concourse.bass_utils import run_bass_kernel_spmd

D = 1024  # d_model
N = 2048  # sequence length
B = 2  # batch
NHEADS = 16
DK = 64
NCORES = 8
GROUPS = 4  # head-groups across cores
HPG = NHEADS // GROUPS  # 4 heads per group
CH = HPG * DK  # 256 channels per group
KT = D // 128  # 8 contraction tiles for the projections
MS = N // 128  # 16 m-slices (key dim)
MP = MS // 2  # 8 m-slice pairs (DoubleRow Ko=2)
NCHUNK = 512  # query-chunk width
NCHUNKS = N // NCHUNK
VPITCH = 72  # per-head pitch in v4p (65 used, pad so ko-stride % 16 == 0)

F32 = mybir.dt.float32
F16 = mybir.dt.float16
BF16 = mybir.dt.bfloat16
FP8 = mybir.dt.float8e4


def _build_bass():
    nc = bacc.Bacc()

    # weights AND x arrive host-prepacked in the exact SBUF tile layout so
    # each loads as ONE contiguous 2D DMA (128 descriptors x 4-8KB) instead
    # of a 1KB-run scatter that grinds the DMA engines for ~6-9us.
    # xT2 layout: [p, (blk, k, j)] = x^T[k*128+p, blk*512+j] - each 512-col
    # query/key block of ALL k-tiles is one contiguous 8KB-per-partition DMA.
    xT_d = nc.dram_tensor("xT", [128, KT * N], BF16, kind="ExternalInput")
    wqT_d = nc.dram_tensor("wqT", [128, KT * CH], BF16, kind="ExternalInput")
    wkT_d = nc.dram_tensor("wkT", [128, KT * CH], BF16, kind="ExternalInput")
    wvT_d = nc.dram_tensor("wvT", [128, KT * CH], BF16, kind="ExternalInput")
    woT_d = nc.dram_tensor("woT", [128, 2 * D], BF16, kind="ExternalInput")
    bb_d = nc.dram_tensor("bb", [128, 4], F32, kind="ExternalInput")
    bv_d = nc.dram_tensor("bv", [128, CH], F32, kind="ExternalInput")
    # f16 output (10 mantissa bits, ~4x tighter than bf16; values << f16
    # range): halves the 8MB/core output DMA.
    yT_d = nc.dram_tensor("yT", [D, N], F16, kind="ExternalOutput")

    with tile.TileContext(nc) as tc:
        with (
            tc.tile_pool(name="persist", bufs=1) as persist,
            tc.tile_pool(name="et_pool", bufs=8) as et_pool,
            tc.tile_pool(name="osb_pool", bufs=1) as osb_pool,
            tc.tile_pool(name="small", bufs=2) as small,
            tc.tile_pool(name="aux_ps", bufs=2, space="PSUM") as aux_ps,
            tc.tile_pool(name="st_ps", bufs=2, space="PSUM") as st_pool,
            tc.tile_pool(name="ot_ps", bufs=1, space="PSUM") as ot_pool,
        ):
            # ---- persistent tiles ----
            # x^T in block-major layout [128, (blk, k, 512)]: each 512-col
            # block of all 8 k-tiles is contiguous, so it loads as ONE
            # dma_start with 8KB-per-partition descriptors, and every reader
            # (projection chains read [k, block]-aligned 512-col spans, V
            # reads 128-col spans inside one block) stays a contiguous slice.
            xtall = persist.tile([128, KT * N], BF16, tag="xtall", name="xtall")

            def xts(k, n0, w):
                blk, j = divmod(n0, 512)
                assert j + w <= 512
                base = blk * (KT * 512) + k * 512 + j
                return xtall[:, base : base + w]
            wkall = persist.tile([128, KT * CH], BF16, tag="wkall", name="wkall")
            wqall = persist.tile([128, KT * CH], BF16, tag="wqall", name="wqall")
            wvall = persist.tile([128, KT * CH], BF16, tag="wvall", name="wvall")
            wotall = persist.tile([128, 2 * D], BF16, tag="wotall", name="wotall")
            bball = persist.tile([128, 4], F32, tag="bball", name="bball")
            bvb = persist.tile([128, CH], F32, tag="bvb", name="bvb")
            qt = [persist.tile([128, N], BF16, tag=f"qt{cs}", name=f"qt{cs}") for cs in range(CH // 128)]
            kt = [persist.tile([128, N], BF16, tag=f"kt{cs}", name=f"kt{cs}") for cs in range(CH // 128)]
            # v4p[mp]: fp8, layout [128, (ko=2, h=4, VPITCH)]; per head cols
            # h*VPITCH .. +64 = V channels, col 64 = ones (denominator trick)
            v4p = [persist.tile([128, 2 * HPG * VPITCH], FP8, tag=f"v4p{mp}", name=f"v4p{mp}") for mp in range(MP)]
            osb = {}
            for c in range(NCHUNKS):
                for cs in range(CH // 128):
                    osb[(c, cs)] = osb_pool.tile(
                        [128, NCHUNK], BF16, tag=f"osb{c}_{cs}", name=f"osb{c}_{cs}"
                    )
            wq = [wqall[:, k * CH : (k + 1) * CH] for k in range(KT)]
            wk = [wkall[:, k * CH : (k + 1) * CH] for k in range(KT)]
            wv = [wvall[:, k * CH : (k + 1) * CH] for k in range(KT)]
            wot = [wotall[:, 0:D], wotall[:, D : 2 * D]]
            bq_t = [bball[:, 0:1], bball[:, 1:2]]
            bk_t = [bball[:, 2:3], bball[:, 3:4]]

            # ---- gpsimd: memsets first (PE warm src, ACT warm src, v4p ones
            # columns), then its share of input DMA issue ----
            dum = persist.tile([128, 640], BF16, tag="dum", name="dum")
            nc.gpsimd.memset(dum, 0.0)
            warm = persist.tile([1, 2], F32, tag="warm", name="warm")
            nc.gpsimd.memset(warm[:, 0:1], 0.0)
            for mp in range(MP):
                v4v = v4p[mp].rearrange("p (k h s) -> p k h s", k=2, h=HPG)
                nc.gpsimd.memset(v4v[:, :, :, 64:65], 1.0)

            # warm the ACT exp table (~2.7us load) under the input DMAs
            nc.scalar.activation(out=warm[:, 1:2], in_=warm[:, 0:1],
                                 func=mybir.ActivationFunctionType.Exp)

            # ---- input DMA issue: ONE queue, strict need-order. The 16 DMA
            # engines drain a queue's descriptors FIFO, so a single queue at
            # full HBM bandwidth delivers tensors in exactly this order;
            # spreading across queues made late-needed blocks steal bandwidth
            # from wq/b0 (round-robin) and pushed the first exp out by ~8us.
            BLK = KT * 512  # columns per x block

            def xblk(b):
                return (xtall[:, b * BLK : (b + 1) * BLK],
                        xT_d[:, b * BLK : (b + 1) * BLK])

            nc.sync.dma_start(out=wkall, in_=wkT_d[:, :])
            nc.sync.dma_start(out=xblk(0)[0], in_=xblk(0)[1])
            nc.sync.dma_start(out=wqall, in_=wqT_d[:, :])
            nc.sync.dma_start(out=wvall, in_=wvT_d[:, :])
            nc.sync.dma_start(out=bball, in_=bb_d[:, :])
            nc.sync.dma_start(out=xblk(1)[0], in_=xblk(1)[1])
            nc.sync.dma_start(out=bvb, in_=bv_d[:, :])
            nc.sync.dma_start(out=xblk(2)[0], in_=xblk(2)[1])
            nc.sync.dma_start(out=xblk(3)[0], in_=xblk(3)[1])
            nc.sync.dma_start(out=wotall, in_=woT_d[:, :])

            # ---- PE p-state warm-up: dummy 512-col matmuls bridge PE from
            # t~=0 until the first x block lands (~11us) so the real chains
            # start at 2.4GHz and the ramp never resets ----
            for _ in range(20):
                dps = aux_ps.tile([128, 512], F32, tag="aux", name="aux_ps_t")
                nc.tensor.matmul(dps, dum[:, 0:128], dum[:, 128:640], start=True, stop=True)

            # ---- filler emitters ----
            def emit_v(ms):
                mp, ko = divmod(ms, 2)
                ps = aux_ps.tile([128, 512], F32, tag="aux", name="aux_ps_t")
                for k in range(KT):
                    nc.tensor.matmul(
                        ps[:, 0:CH],
                        xts(k, ms * 128, 128),
                        wv[k],
                        start=(k == 0),
                        stop=(k == KT - 1),
                    )
                v4v = v4p[mp].rearrange("p (k h s) -> p k h s", k=2, h=HPG)
                nc.vector.tensor_add(
                    out=v4v[:, ko, :, 0:64],
                    in0=ps[:, 0:CH].rearrange("p (h c) -> p h c", c=64),
                    in1=bvb.rearrange("p (h c) -> p h c", c=64),
                )

            def emit_qk_chain(isq, cs, n0):
                dst, w, bias = (qt, wq, bq_t) if isq else (kt, wk, bk_t)
                ps = aux_ps.tile([128, 512], F32, tag="aux", name="aux_ps_t")
                for k in range(KT):
                    nc.tensor.matmul(
                        ps,
                        w[k][:, cs * 128 : (cs + 1) * 128],
                        xts(k, n0, 512),
                        start=(k == 0),
                        stop=(k == KT - 1),
                    )
                nc.vector.tensor_scalar_add(
                    out=dst[cs][:, n0 : n0 + 512], in0=ps, scalar1=bias[cs]
                )

            # split Q/K chain: two INDEPENDENT 4-matmul groups on separate
            # psum banks, merged on DVE - a filler unit then interrupts the
            # score stream by ~0.9us instead of ~1.8us.
            qkpend = {}

            def emit_qk_half(isq, cs, n0, half):
                dst, w, bias = (qt, wq, bq_t) if isq else (kt, wk, bk_t)
                ps = aux_ps.tile([128, 512], F32, tag="aux", name="aux_qk_t")
                for k in range(half * 4, half * 4 + 4):
                    nc.tensor.matmul(
                        ps,
                        w[k][:, cs * 128 : (cs + 1) * 128],
                        xts(k, n0, 512),
                        start=(k == half * 4),
                        stop=(k == half * 4 + 3),
                    )
                if half == 0:
                    t = small.tile([128, 512], F32, tag="qkstash", name="qkstash_t", bufs=2)
                    nc.vector.tensor_copy(out=t, in_=ps)
                    qkpend[(isq, cs, n0)] = t
                else:
                    tsum = small.tile([128, 512], F32, tag="qksum", name="qksum_t", bufs=2)
                    nc.vector.tensor_add(out=tsum, in0=ps, in1=qkpend.pop((isq, cs, n0)))
                    nc.vector.tensor_scalar_add(
                        out=dst[cs][:, n0 : n0 + 512], in0=tsum, scalar1=bias[cs]
                    )

            # O-projection: both cs halves accumulate on ONE psum bank, one
            # DVE drain (copy+convert to f16), DMA out from the sync queue.
            def emit_fab(c, msl):
                yp = aux_ps.tile([128, 512], F32, tag="aux", name="aux_fab_t")
                nc.tensor.matmul(
                    yp, wot[0][:, msl * 128 : (msl + 1) * 128], osb[(c, 0)],
                    start=True, stop=False,
                )
                nc.tensor.matmul(
                    yp, wot[1][:, msl * 128 : (msl + 1) * 128], osb[(c, 1)],
                    start=False, stop=True,
                )
                ysb = small.tile([128, 512], F16, tag="ysb", name="ysb_t", bufs=4)
                nc.vector.tensor_copy(out=ysb, in_=yp)
                nc.sync.dma_start(
                    out=yT_d[msl * 128 : (msl + 1) * 128, c * NCHUNK : (c + 1) * NCHUNK],
                    in_=ysb,
                )

            # ---- prelude: just what segment 0 needs to start ----
            emit_qk_chain(False, 0, 0)  # kt[0] cols 0:512 (ms 0..3)
            emit_qk_chain(True, 0, 0)   # qt[0] cols 0:512 (chunk 0)

            # ---- deadline-paced fillers, keyed by (segment, slot) ----
            def V(ms):
                return lambda: emit_v(ms)

            def QKH(isq, cs, n0, half):
                return lambda: emit_qk_half(isq, cs, n0, half)

            def FAB(c, msl):
                return lambda: emit_fab(c, msl)

            fstash = {}

            def emit_fa3(msl):
                yp = aux_ps.tile([128, 512], F32, tag="aux", name="aux_fab_t")
                nc.tensor.matmul(
                    yp, wot[0][:, msl * 128 : (msl + 1) * 128], osb[(3, 0)],
                    start=True, stop=True,
                )
                t = small.tile([128, 512], F32, tag="fstash", name="fstash_t", bufs=4)
                nc.vector.tensor_copy(out=t, in_=yp)
                fstash[msl] = t

            def FA3(msl):
                return lambda: emit_fa3(msl)

            seg_fill = {
                # A*V(mp) executes at deferral pop (pair mp+3, or +2 at a
                # segment seam), so V(14)/V(15) may spill into seg 1 slots
                # 0/1 (v4p[7] is read at seg-1 pair-1 flush) - lightening
                # the PE-bound seg 0 by ~1.7us.
                0: {
                    0: [V(0)], 1: [V(1)],
                    2: [V(2), QKH(False, 0, 512, 0)], 3: [V(3), QKH(False, 0, 512, 1)],
                    4: [V(4)], 5: [V(5)],
                    6: [V(6), QKH(False, 0, 1024, 0)], 7: [V(7), QKH(False, 0, 1024, 1)],
                    8: [V(8)], 9: [V(9)],
                    10: [V(10), QKH(False, 0, 1536, 0)], 11: [V(11), QKH(False, 0, 1536, 1)],
                    12: [V(12)], 13: [V(13)],
                    14: [QKH(True, 0, 512, 0)], 15: [QKH(True, 0, 512, 1)],
                },
                1: {
                    0: [V(14)], 1: [V(15)],
                    2: [QKH(True, 0, 1024, 0)], 3: [QKH(True, 0, 1024, 1)],
                    4: [QKH(False, 1, 0, 0)], 5: [QKH(False, 1, 0, 1)],
                    6: [QKH(False, 1, 512, 0)], 7: [QKH(False, 1, 512, 1)],
                    8: [QKH(True, 0, 1536, 0)], 9: [QKH(True, 0, 1536, 1)],
                },
                2: {
                    1: [QKH(False, 1, 1024, 0)], 2: [QKH(False, 1, 1024, 1)],
                    3: [QKH(False, 1, 1536, 0)], 4: [QKH(False, 1, 1536, 1)],
                },
                3: {
                    1: [QKH(True, 1, 0, 0)], 2: [QKH(True, 1, 0, 1)],
                    3: [QKH(True, 1, 512, 0)], 4: [QKH(True, 1, 512, 1)],
                    5: [QKH(True, 1, 1024, 0)], 6: [QKH(True, 1, 1024, 1)],
                    7: [QKH(True, 1, 1536, 0)], 8: [QKH(True, 1, 1536, 1)],
                },
                4: {},
                5: {s: [FAB(0, s - 8)] for s in range(8, 16)},
                6: {s: [FAB(1, s - 8)] for s in range(8, 16)},
                # seg 7 also pre-runs 4 of chunk 3's cs0-half units (they
                # only need osb[(3,0)]) so the epilogue after the final
                # normalize is half as long.
                7: {s: [FAB(2, s - 8)] + ([FA3((s - 8) // 2)] if s % 2 == 0 else [])
                    for s in range(8, 16)},
            }

            # ---- attention: pair-major segments, A*V + normalize deferred
            # by one pair so PE never waits on ScalarE ----
            ot_store = {}
            deferred = []

            def make_av(seg, cs, mp, et):
                def go():
                    if seg not in ot_store:
                        ot_store[seg] = [
                            ot_pool.tile([65, NCHUNK], F32, tag=f"ot{hi}", name=f"ot{hi}_t")
                            for hi in range(2)
                        ]
                    ot = ot_store[seg]
                    etv = et.rearrange("p (k n) -> p k n", k=2)
                    v4v = v4p[mp].rearrange("p (k s) -> p k s", k=2)
                    for hi in range(2):
                        h = 2 * cs + hi
                        nc.tensor.matmul(
                            ot[hi],
                            v4v[:, :, h * VPITCH : h * VPITCH + 65],
                            etv[:, :, hi * 512 : (hi + 1) * 512],
                            start=(mp == 0),
                            stop=(mp == MP - 1),
                            perf_mode=mybir.MatmulPerfMode.DoubleRow,
                        )
                return go

            def make_norm(seg, c, cs):
                last = seg == 7

                def go():
                    ot = ot_store.pop(seg)
                    # fast PSUM drain first (frees ot for the next segment's
                    # A*V), then the reciprocal chain runs off critical path.
                    # reciprocal_approx_fast: ~18 correct bits, ~5x faster
                    # than the exact multi-pass DVE reciprocal; denominators
                    # are sums of >=2048 positive exps, far from any edge
                    # case.
                    oraw = []
                    for hi in range(2):
                        if last:  # ot is never reused: copy only row 64
                            t = small.tile([65, NCHUNK], F32, tag="oraw", name="oraw_t", bufs=2)
                            nc.vector.tensor_copy(out=t[64:65, :], in_=ot[hi][64:65, :])
                        else:
                            t = small.tile([65, NCHUNK], F32, tag="oraw", name="oraw_t", bufs=2)
                            nc.vector.tensor_copy(out=t, in_=ot[hi])
                        oraw.append(t)
                    # transpose the denominator row to [128,4] via SBUF-SBUF
                    # DMA so the exact DVE reciprocal costs ~free-size-4
                    # cycles (a [1,512] reciprocal costs 3.3us), then
                    # transpose back and broadcast across partitions.
                    rb = []
                    for hi in range(2):
                        q = nc.scalar if (last and hi == 1) else nc.sync
                        rcin = small.tile([128, NCHUNK // 128], F32, tag="rcin", name="rcin_t", bufs=2)
                        q.dma_start(out=rcin, in_=oraw[hi][64:65, :])
                        rc = small.tile([128, NCHUNK // 128], F32, tag="rc", name="rc_t", bufs=2)
                        nc.vector.reciprocal(out=rc, in_=rcin)
                        rflat = small.tile([1, NCHUNK], F32, tag="rflat", name="rflat_t", bufs=2)
                        q.dma_start(out=rflat, in_=rc)
                        rbt = small.tile([64, NCHUNK], F32, tag="rb", name="rb_t", bufs=2)
                        nc.gpsimd.partition_broadcast(rbt, rflat)
                        rb.append(rbt)
                    for hi in range(2):
                        nc.vector.tensor_mul(
                            out=osb[(c, cs)][hi * 64 : (hi + 1) * 64, :],
                            in0=(ot[hi] if last else oraw[hi])[0:64, :],
                            in1=rb[hi],
                        )
                return go

            seg = 0
            for cs in range(2):
                for c in range(NCHUNKS):
                    n0 = c * NCHUNK
                    fillers = seg_fill[seg]
                    for mp in range(MP):
                        et = et_pool.tile([128, 2048], FP8, tag="et", name="et_t")
                        # both slices' scores+exps emit BEFORE the pair's
                        # fillers: the PE queue then always has the next
                        # scores ahead of filler bunches, so exp(2p+1) can
                        # never be starved by a V/QK chain (the st double
                        # buffer already allows this one-slot lead).
                        for mi in range(2):
                            ms = 2 * mp + mi
                            st = st_pool.tile([128, 1024], F32, tag="st", name="st_t")
                            for hi in range(2):
                                r0 = hi * 64
                                nc.tensor.matmul(
                                    st[:, hi * 512 : (hi + 1) * 512],
                                    kt[cs][r0 : r0 + 64, ms * 128 : (ms + 1) * 128],
                                    qt[cs][r0 : r0 + 64, n0 : n0 + 512],
                                    start=True,
                                    stop=True,
                                )
                            nc.scalar.activation(
                                out=et[:, mi * 1024 : (mi + 1) * 1024],
                                in_=st,
                                func=mybir.ActivationFunctionType.Exp,
                                scale=float(1.0 / np.sqrt(DK)),
                            )
                            if mi == 0:
                                # keep 2 items deferred: A*V(p) runs ~2 pairs
                                # after its exps, so the in-order PE queue
                                # never waits on ScalarE (flushing to depth 1
                                # measured 0.67us PE->exp stalls every pair)
                                while len(deferred) > 2:
                                    deferred.pop(0)()
                        for mi in range(2):
                            for f in fillers.get(2 * mp + mi, []):
                                f()
                        deferred.append(make_av(seg, cs, mp, et))
                    deferred.append(make_norm(seg, c, cs))
                    seg += 1
            # epilogue: flush the final A*V + normalize FIRST (the PE
            # wait-queue lets later ready matmuls bypass the two parked
            # A*Vs), then complete the 4 pre-run cs0 halves with cs1
            # matmul + DVE add, then 4 fused both-half units.
            while deferred:
                deferred.pop(0)()
            for msl in range(4):
                yp = aux_ps.tile([128, 512], F32, tag="aux", name="aux_fab_t")
                nc.tensor.matmul(
                    yp, wot[1][:, msl * 128 : (msl + 1) * 128], osb[(3, 1)],
                    start=True, stop=True,
                )
                ysb = small.tile([128, 512], F16, tag="ysb", name="ysb_t", bufs=4)
                nc.vector.tensor_add(out=ysb, in0=yp, in1=fstash.pop(msl))
                nc.sync.dma_start(
                    out=yT_d[msl * 128 : (msl + 1) * 128, 3 * NCHUNK : 4 * NCHUNK],
                    in_=ysb,
                )
            for msl in range(4, D // 128):
                emit_fab(3, msl)
    nc.compile()
    return nc


_NC = None


def _get_nc():
    global _NC
    if _NC is None:
        _NC = _build_bass()
    return _NC


def build_in_maps(inputs):
    x = np.asarray(inputs["x"], dtype=np.float32)
    W_Q = np.asarray(inputs["W_Q"], dtype=np.float32)
    W_K = np.asarray(inputs["W_K"], dtype=np.float32)
    W_V = np.asarray(inputs["W_V"], dtype=np.float32)
    W_O = np.asarray(inputs["W_O"], dtype=np.float32)
    b_Q = np.asarray(inputs["b_Q"], dtype=np.float32)
    b_K = np.asarray(inputs["b_K"], dtype=np.float32)
    b_V = np.asarray(inputs["b_V"], dtype=np.float32)

    def pack_w(WT):  # [D, CH] -> [128, KT*CH], tile [p, k*CH+c] = WT[k*128+p, c]
        return np.ascontiguousarray(
            WT.reshape(KT, 128, CH).transpose(1, 0, 2).reshape(128, KT * CH)
            .astype(ml_dtypes.bfloat16)
        )

    in_maps = []
    for core in range(NCORES):
        b, g = divmod(core, GROUPS)
        sl = slice(g * CH, (g + 1) * CH)
        woT = W_O[:, sl].T  # [CH, D]
        bb = np.stack(
            [b_Q[sl][:128], b_Q[sl][128:], b_K[sl][:128], b_K[sl][128:]], axis=1
        )
        # block-major x^T: [p, (blk, k, j)] = x^T[k*128+p, blk*512+j]
        xT2 = (
            x[b].T.reshape(KT, 128, NCHUNKS, 512)
            .transpose(1, 2, 0, 3)
            .reshape(128, KT * N)
        )
        in_maps.append(
            {
                "xT": np.ascontiguousarray(xT2.astype(ml_dtypes.bfloat16)),
                "wqT": pack_w(W_Q[sl, :].T),
                "wkT": pack_w(W_K[sl, :].T),
                "wvT": pack_w(W_V[sl, :].T),
                "woT": np.ascontiguousarray(
                    woT.reshape(2, 128, D).transpose(1, 0, 2).reshape(128, 2 * D)
                    .astype(ml_dtypes.bfloat16)
                ),
                "bb": np.ascontiguousarray(bb),
                "bv": np.ascontiguousarray(
                    np.broadcast_to(b_V[sl], (128, CH)).astype(np.float32)
                ),
            }
        )
    return in_maps


def kernel(**inputs):
    in_maps = build_in_maps(inputs)
    nc = _get_nc()
    res = run_bass_kernel_spmd(nc, in_maps, core_ids=list(range(NCORES)))

    b_O = np.asarray(inputs["b_O"], dtype=np.float32)
    out = np.zeros((B, N, D), dtype=np.float32)
    for core in range(NCORES):
        b = core // GROUPS
        out[b] += res.results[core]["yT"].T
    out += b_O
    return out
